# revision 22
# baseline (speedup 1.0000x reference)
"""HardTripletLoss on 8 Trainium2 NeuronCores (Bass/Tile), fp8 edition.

Math
----
reference: emb = l2_normalize(embeddings); dist = cdist(emb, emb);
  pos_stat[i] = mean_{j: same class, j!=i} dist[i,j]
  neg_stat[i] = min_{j: diff class} dist[i,j]
  loss = mean over valid rows of relu(pos_stat - neg_stat + 1)

For unit vectors dist^2 = 2 - 2*ghat with ghat = N @ N.T.  We quantize
X = 8*N to fp8e4m3 (measured end-to-end rel err ~5e-4) and fold the
class mask into the GEMM: P = X@X.T - 128*S = 64*ghat - 128*S, with the
-128*S term contributed by a small one-hot block (lhsT 16*Y, rhs -8*Y,
Y = onehot(labels)).  Then per row:
  positive dists = sqrt(relu(-P/32 - 2))     (diff-class and diagonal -> 0)
  hardest negative^2 = min over row of e, e = 2 - P/32 = dist^2 + 4*S

Host-side trick: rows are SORTED BY LABEL, so each row's same-class
columns live in a narrow diagonal band.  The sqrt/accumulate positive
pass (ACT engine) then only touches a ~(128+2*maxc)-wide column window
per 128-row m-tile instead of all 4096 columns, and the one-hot GEMM
blocks are only emitted for the 2-3 slabs that windows touch.  Sorting
is a symmetric permutation of dist: per-row stats permute with it and
the final mean is unchanged.

Sharding: rows split 512/core (data parallel).  Every core holds all
4096 columns as 8 fp8 slabs of 512 cols; slab order is rotated per core
so slab 0 contains its own shard columns (the matmul stationary
operand) and the label-sorted diagonal windows sit at a core-invariant
position - one SPMD program for all 8 cores.

GEMM runs in fp8 DoubleRow perf mode: each matmul contracts TWO
128-row k-chunks at 0.5 cycles/row - measured 215ns per [128,512]
DoubleRow matmul vs 259ns per half-the-work bf16 matmul.  Chunks are
processed column-pair-major so slab j is first needed ~j/8 of the way
through the GEMM, hiding the HBM stream behind compute (the bf16
baseline was DMA-bound: 5.3MB/core at ~95GB/s; fp8 needs 2.2MB).

The hardest-negative reduction is split across ACT and DVE (DVE may
read only ONE PSUM operand per instruction, so it ingests PSUM at just
1 elem/cycle; TENSOR_REDUCE never engages a packed fast mode on HW,
but fp16 TENSOR_TENSOR runs 2x): 8 of 16 PSUM chunks get a direct DVE
tensor_reduce(max of P); m-tiles 0 and 1 instead get an ACT pass
e = 2 - P/32 (monotone decreasing, = dist^2 + 4S) written as fp16 to
SBUF, folded by per-m-tile fp16 TT-min chains on DVE at 2 elem/cycle
with one 1x final reduce each.  The label-sort windows then read their
positive dist^2 directly off the fp16 e values (relu(e - 4)) where
available.  Host combines both forms, plus the windowed positive sums,
into the loss.  Engine loads balance at ~15.5us each (PE / ACT / DVE)
with the 2.2MB HBM stream fully hidden; measured wall time varies with
the chip's DVFS throttle state.
"""

import sys

if "/opt/trn_rl_repo" not in sys.path:
    sys.path.insert(0, "/opt/trn_rl_repo")

import ml_dtypes
import numpy as np

import concourse.bass as bass
import concourse.bacc as bacc
import concourse.mybir as mybir
import concourse.tile as tile
from concourse.bass_utils import run_bass_kernel_spmd

F32 = mybir.dt.float32
F16 = mybir.dt.float16
FP8 = mybir.dt.float8e4
NP_FP8 = ml_dtypes.float8_e4m3
ALU = mybir.AluOpType
ACTF = mybir.ActivationFunctionType
AXX = mybir.AxisListType.X
PERF = mybir.MatmulPerfMode.DoubleRow

B = 4096
D = 512
C = 64
NCORES = 8
SHARD = B // NCORES          # 512 rows per core
MT = 4                       # m-tiles per core
NJ = 8                       # column slabs of 512
KC = 4                       # data k-chunks of 128 (contracted in 2 pairs)
SCALE = 8.0                  # fp8 input scale; P = 64*ghat - 128*S
# (c, m) psum chunks routed through the ACT e-pass (e = 2 - P/32, fp16
# SBUF) instead of a direct DVE max; per-m TT-min chains on DVE then
# fold the fp16 e-chunks at 2 elem/cycle
ACT_MAX_CHUNKS = {
    (0, 0), (1, 0), (2, 0), (3, 0),
    (0, 1), (1, 1), (2, 1), (3, 1),
}

MARGIN = 1.0


def _plan(maxc):
    """Label-sorted geometry, in LOCAL (rotated) column coords.

    Window of m-tile m = all columns that can share a class with its
    rows: [128m - (maxc-1), 128m + 127 + maxc).  Returns the ACT
    positive-pass segments per 1024-col psum chunk and the (m, slab)
    pairs needing a one-hot matmul.
    """
    wins = []
    for m in range(MT):
        lo = 128 * m - (maxc - 1)
        hi = 128 * m + 128 + (maxc - 1)
        w = min(hi - lo, B)
        ivs = []
        lo %= B
        while w > 0:
            take = min(w, B - lo)
            ivs.append((lo, lo + take))
            lo = 0
            w -= take
        wins.append(ivs)

    segs = []    # (m, c, lo_in_chunk, hi_in_chunk)
    oh = set()   # (m, local slab)
    for m, ivs in enumerate(wins):
        for (a, b) in ivs:
            for c in range(4):
                clo, chi = 1024 * c, 1024 * (c + 1)
                s_lo, s_hi = max(a, clo), min(b, chi)
                if s_lo < s_hi:
                    segs.append((m, c, s_lo - clo, s_hi - clo))
            for s in range(NJ):
                if max(a, 512 * s) < min(b, 512 * (s + 1)):
                    oh.add((m, s))
    oh_slabs = sorted({s for (_, s) in oh})
    return segs, oh, oh_slabs


def _build_nc(maxc):
    segs, oh, oh_slabs = _plan(maxc)
    noh = len(oh_slabs)
    oh_idx = {s: t for t, s in enumerate(oh_slabs)}
    nstat = 16 + len(segs)

    nc = bacc.Bacc(
        "TRN2",
        target_bir_lowering=False,
        debug=False,
        enable_asserts=False,
        num_devices=NCORES,
    )
    atp0 = nc.dram_tensor("atp0", [128, KC, 512], FP8, kind="ExternalInput")
    atp1 = nc.dram_tensor("atp1", [128, KC, 512], FP8, kind="ExternalInput")
    atp23 = nc.dram_tensor("atp23", [128, 2, KC, 512], FP8, kind="ExternalInput")
    atp4567 = nc.dram_tensor(
        "atp4567", [128, 2, 2, KC, 512], FP8, kind="ExternalInput"
    )
    ylr = nc.dram_tensor("ylr", [32, 1 + noh, 2, 512], FP8, kind="ExternalInput")
    stats_d = nc.dram_tensor("stats", [128, nstat], F32, kind="ExternalOutput")

    with tile.TileContext(nc) as tc:
        with (
            tc.tile_pool(name="slabs", bufs=1) as slabs,
            tc.tile_pool(name="psum", bufs=4, space=bass.MemorySpace.PSUM) as psum,
            tc.tile_pool(name="scr", bufs=1) as scr,
            tc.tile_pool(name="esb", bufs=10) as esbp,
            tc.tile_pool(name="chn", bufs=8) as chnp,
            tc.tile_pool(name="stat", bufs=1) as stat,
        ):
            # --- SBUF tiles -------------------------------------------------
            s0 = slabs.tile([128, KC, 512], FP8, name="s0", tag="s0")
            s1 = slabs.tile([128, KC, 512], FP8, name="s1", tag="s1")
            s23 = slabs.tile([128, 2, KC, 512], FP8, name="s23", tag="s23")
            s47 = slabs.tile([128, 2, 2, KC, 512], FP8, name="s47", tag="s47")
            ylrt = stat.tile([32, 1 + noh, 2, 512], FP8, name="ylrt", tag="ylrt")
            parts = stat.tile([128, nstat], F32, name="parts", tag="parts")
            # ACT window scratch
            wt = scr.tile([128, 1024], F32, name="wt", tag="wt")
            dsc = scr.tile([128, 1024], F32, name="dsc", tag="dsc")

            # --- DMA issue.  The scalar (Activation) HWDGE queue measures
            # ~2x the sync queue's bandwidth and throughput scales with the
            # per-partition contiguous run, so tensors are merged into large
            # rows: early pieces stream on scalar, the late 1MB on sync.
            nc.sync.dma_start(ylrt[:], ylr.ap())
            nc.sync.dma_start(s0[:], atp0.ap())
            nc.sync.dma_start(s1[:], atp1.ap())
            nc.sync.dma_start(s23[:], atp23.ap())
            nc.scalar.dma_start(s47[:], atp4567.ap())

            # --- constants & warm-up ---------------------------------------
            bias_c = {}
            for bname, bval in [("m2", -2.0), ("p2", 2.0), ("m4", -4.0), ("z", 0.0)]:
                bt = stat.tile([128, 1], F32, name=f"bc_{bname}", tag=f"bc_{bname}")
                nc.gpsimd.memset(bt[:], bval)
                bias_c[bname] = bt

            # Sqrt first: pulls in the one table set that serves both
            # Sqrt and Relu, so only a single ACT_TABLE_LOAD is paid
            warm = stat.tile([128, 1], F32, name="warm", tag="warm")
            nc.scalar.activation(warm[:], bias_c["z"][:], ACTF.Sqrt,
                                 bias=bias_c["z"][:])
            nc.scalar.activation(warm[:], warm[:], ACTF.Relu,
                                 bias=bias_c["z"][:])

            # PE warm-up: dummy matmuls open the HAM clock gate / p-state
            # ramp while the first slab DMA is in flight
            warm_w = stat.tile([128, 2, 128], FP8, name="warm_w", tag="warm_w")
            warm_x = stat.tile([128, 2, 512], FP8, name="warm_x", tag="warm_x")
            nc.gpsimd.memset(warm_w[:], 0.0)
            nc.gpsimd.memset(warm_x[:], 0.0)
            wpt = psum.tile([128, 512], F32, name="wpt", tag="pt")
            for _ in range(5):
                nc.tensor.matmul(
                    wpt[:], warm_w[:], warm_x[:], start=True, stop=True,
                    perf_mode=PERF,
                )

            # --- main loop: column-pair-major over (chunk, m-tile) ----------
            def rhs_ap(s, kk):
                if s < 2:
                    return (s0, s1)[s][:, 2 * kk : 2 * kk + 2, :]
                if s < 4:
                    return s23[:, s - 2, 2 * kk : 2 * kk + 2, :]
                return s47[:, (s - 4) // 2, (s - 4) % 2, 2 * kk : 2 * kk + 2, :]

            segcol = {}
            for i, (m, c, lo, hi) in enumerate(segs):
                segcol[(m, c, lo, hi)] = 16 + i

            chain = [None] * MT
            last_act_c = {}
            for (c, m) in ACT_MAX_CHUNKS:
                last_act_c[m] = max(last_act_c.get(m, -1), c)

            def emit_region(pt, c, m, sj):
                s = 2 * c + sj
                for kk in range(2):
                    last = kk == 1 and (m, s) not in oh
                    nc.tensor.matmul(
                        pt[:, sj * 512 : (sj + 1) * 512],
                        s0[:, 2 * kk : 2 * kk + 2, m * 128 : (m + 1) * 128],
                        rhs_ap(s, kk),
                        start=(kk == 0),
                        stop=last,
                        perf_mode=PERF,
                    )
                if (m, s) in oh:
                    nc.tensor.matmul(
                        pt[:, sj * 512 : (sj + 1) * 512],
                        ylrt[:, 0, :, m * 128 : (m + 1) * 128],
                        ylrt[:, 1 + oh_idx[s], :, :],
                        start=False,
                        stop=True,
                        perf_mode=PERF,
                    )

            c0_pts = {}
            for c in range(4):
                for m in range(MT):
                    if c == 0:
                        # slab-0 halves of all four m-tiles run first: a
                        # ~2.6us PE runway while slab 1 is still streaming
                        if m == 0:
                            for m_ in range(MT):
                                c0_pts[m_] = psum.tile(
                                    [128, 1024], F32, name="pt", tag="pt"
                                )
                                emit_region(c0_pts[m_], 0, m_, 0)
                        pt = c0_pts[m]
                        emit_region(pt, 0, m, 1)
                    else:
                        pt = psum.tile([128, 1024], F32, name="pt", tag="pt")
                        for sj in range(2):
                            emit_region(pt, c, m, sj)
                    # hardest-negative reduction, split by chunk route
                    et = None
                    if (c, m) in ACT_MAX_CHUNKS:
                        # ACT: e = 2 - P/32 = dist^2 + 4S to fp16 SBUF;
                        # DVE folds it into the m-tile's TT-min chain at
                        # 2 elem/cycle, one 1x final reduce per m-tile
                        et = esbp.tile([128, 1024], F16, name="et", tag="et")
                        nc.scalar.activation(
                            et[:], pt[:], ACTF.Relu,
                            bias=bias_c["p2"][:], scale=-1.0 / 32.0,
                        )
                        if chain[m] is None:
                            chain[m] = et
                        else:
                            r = chnp.tile([128, 1024], F16, name="rc", tag="rc")
                            nc.vector.tensor_tensor(
                                r[:], chain[m][:], et[:], ALU.min
                            )
                            chain[m] = r
                        if c == last_act_c[m]:
                            nc.vector.tensor_reduce(
                                parts[:, 4 * m : 4 * m + 1], chain[m][:],
                                axis=AXX, op=ALU.min,
                            )
                    else:
                        # DVE direct: rowmax(P) from PSUM
                        mcol = 4 * m + c
                        nc.vector.tensor_reduce(
                            parts[:, mcol : mcol + 1], pt[:],
                            axis=AXX, op=ALU.max,
                        )
                    # positive pass: dist = sqrt(relu(-P/32 - 2)) over the
                    # diagonal window (= sqrt(relu(e - 4)) on the e-path);
                    # accum_out emits the row-sum for free
                    for (m_, c_, lo, hi) in segs:
                        if m_ != m or c_ != c:
                            continue
                        w = hi - lo
                        col = segcol[(m_, c_, lo, hi)]
                        if et is not None:
                            nc.scalar.activation(
                                wt[:, :w], et[:, lo:hi], ACTF.Relu,
                                bias=bias_c["m4"][:],
                            )
                        else:
                            nc.scalar.activation(
                                wt[:, :w], pt[:, lo:hi], ACTF.Relu,
                                bias=bias_c["m2"][:], scale=-1.0 / 32.0,
                            )
                        nc.scalar.activation(
                            dsc[:, :w], wt[:, :w], ACTF.Sqrt,
                            bias=bias_c["z"][:],
                            accum_out=parts[:, col : col + 1],
                        )

            nc.scalar.dma_start(stats_d.ap(), parts[:])

    nc.compile()
    return nc, segs, oh_slabs, nstat


_NC_CACHE: dict = {}


def _get_nc(maxc):
    if maxc not in _NC_CACHE:
        _NC_CACHE[maxc] = _build_nc(maxc)
    return _NC_CACHE[maxc]


def _prep_inputs(embeddings: np.ndarray, labels: np.ndarray):
    E = np.asarray(embeddings, dtype=np.float32)
    L = np.asarray(labels).astype(np.int64)
    assert E.shape == (B, D) and L.shape == (B,)

    order = np.argsort(L, kind="stable")
    Ls = L[order]
    nrm = np.maximum(np.linalg.norm(E, axis=1), 1e-12)
    N = (E / nrm[:, None]).astype(np.float32)[order]

    cnt = np.bincount(Ls, minlength=C)
    maxc = int(cnt.max())
    nc, segs, oh_slabs, nstat = _get_nc(maxc)

    X8 = np.ascontiguousarray((SCALE * N).T.astype(NP_FP8))       # [D, B]
    # S[g][p][c][x] = X8[128c + p, 512g + x]
    S = np.ascontiguousarray(
        X8.reshape(KC, 128, NJ, 512).transpose(2, 1, 0, 3)
    )                                                             # [g,p,c,x]
    Y = (Ls[None, :] == np.arange(C, dtype=np.int64)[:, None]).astype(np.float32)

    in_maps = []
    for r in range(NCORES):
        Sr = np.roll(S, -r, axis=0)                               # local j
        rows = slice(SHARD * r, SHARD * (r + 1))
        ylc = np.ascontiguousarray(
            (2 * SCALE * Y[:, rows]).reshape(2, 32, SHARD)
            .transpose(1, 0, 2).astype(NP_FP8)
        )
        yrr = np.stack(
            [
                (-SCALE * Y[:, 512 * ((r + s) % NJ) : 512 * ((r + s) % NJ) + 512])
                .reshape(2, 32, 512)
                for s in oh_slabs
            ]
        )                                                         # [t,h,p,x]
        yrr = np.ascontiguousarray(yrr.transpose(2, 0, 1, 3).astype(NP_FP8))
        in_maps.append(
            {
                "atp0": np.ascontiguousarray(Sr[0]),
                "atp1": np.ascontiguousarray(Sr[1]),
                "atp23": np.ascontiguousarray(Sr[2:4].transpose(1, 0, 2, 3)),
                "atp4567": np.ascontiguousarray(
                    Sr[4:8].reshape(2, 2, 128, KC, 512).transpose(2, 0, 1, 3, 4)
                ),
                "ylr": np.ascontiguousarray(
                    np.concatenate([ylc[:, None, :, :], yrr], axis=1)
                ),
            }
        )

    pos_cnt = cnt[Ls] - 1
    neg_cnt = B - cnt[Ls]
    invc = (1.0 / np.maximum(pos_cnt, 1)).astype(np.float32)
    valid = ((pos_cnt > 0) & (neg_cnt > 0)).astype(np.float32)
    return nc, segs, nstat, in_maps, (invc, valid)


def _finish(results, segs, nstat, aux):
    invc, valid = aux
    pos_sum = np.empty(B, dtype=np.float32)
    neg2 = np.empty(B, dtype=np.float32)
    for r in range(NCORES):
        st = np.asarray(results[r]["stats"])                      # [128, nstat]
        grid = st[:, :16].reshape(128, MT, 4)
        act_ms = {m for (_, m) in ACT_MAX_CHUNKS}
        n2 = np.full((128, MT), np.inf, dtype=np.float32)
        for m in range(MT):
            if m in act_ms:
                # the m-tile's TT-min chain result lands in col 4m+0
                n2[:, m] = np.minimum(n2[:, m], grid[:, m, 0])
            for c in range(4):
                if (c, m) in ACT_MAX_CHUNKS or (c == 0 and m in act_ms):
                    continue
                n2[:, m] = np.minimum(n2[:, m], 2.0 - grid[:, m, c] / 32.0)
        ps = np.zeros((128, MT), dtype=np.float32)
        for i, (m, c, lo, hi) in enumerate(segs):
            ps[:, m] += st[:, 16 + i]
        rows = slice(SHARD * r, SHARD * (r + 1))
        pos_sum[rows] = ps.T.reshape(SHARD)
        neg2[rows] = n2.T.reshape(SHARD)
    pos_stat = pos_sum * invc
    neg_stat = np.sqrt(np.maximum(neg2, 0.0), dtype=np.float32)
    per_row = np.maximum(pos_stat - neg_stat + MARGIN, 0.0) * valid
    n_valid = float(valid.sum())
    total = float(per_row.sum(dtype=np.float32))
    out = total / max(n_valid, 1.0) if n_valid > 0 else 0.0
    return np.array(out, dtype=np.float32)


def kernel(embeddings, labels, _run_kwargs=None):
    nc, segs, nstat, in_maps, aux = _prep_inputs(embeddings, labels)
    res = run_bass_kernel_spmd(
        nc, in_maps, core_ids=list(range(NCORES)), **(_run_kwargs or {})
    )
    out = _finish(res.results, segs, nstat, aux)
    if _run_kwargs:
        return out, res
    return out


# revision 23
# speedup vs baseline: 1.0092x; 1.0092x over previous
"""HardTripletLoss on 8 Trainium2 NeuronCores (Bass/Tile), fp8 edition.

Math
----
reference: emb = l2_normalize(embeddings); dist = cdist(emb, emb);
  pos_stat[i] = mean_{j: same class, j!=i} dist[i,j]
  neg_stat[i] = min_{j: diff class} dist[i,j]
  loss = mean over valid rows of relu(pos_stat - neg_stat + 1)

For unit vectors dist^2 = 2 - 2*ghat with ghat = N @ N.T.  We quantize
X = 8*N to fp8e4m3 (measured end-to-end rel err ~5e-4) and fold the
class mask into the GEMM: P = X@X.T - 128*S = 64*ghat - 128*S, with the
-128*S term contributed by a small one-hot block (lhsT 16*Y, rhs -8*Y,
Y = onehot(labels)).  Then per row:
  positive dists = sqrt(relu(-P/32 - 2))     (diff-class and diagonal -> 0)
  hardest negative^2 = min over row of e, e = 2 - P/32 = dist^2 + 4*S

Host-side trick: rows are SORTED BY LABEL, so each row's same-class
columns live in a narrow diagonal band.  The sqrt/accumulate positive
pass (ACT engine) then only touches a ~(128+2*maxc)-wide column window
per 128-row m-tile instead of all 4096 columns, and the one-hot GEMM
blocks are only emitted for the 2-3 slabs that windows touch.  Sorting
is a symmetric permutation of dist: per-row stats permute with it and
the final mean is unchanged.

Sharding: rows split 512/core (data parallel).  Every core holds all
4096 columns as 8 fp8 slabs of 512 cols; slab order is rotated per core
so slab 0 contains its own shard columns (the matmul stationary
operand) and the label-sorted diagonal windows sit at a core-invariant
position - one SPMD program for all 8 cores.

GEMM runs in fp8 DoubleRow perf mode: each matmul contracts TWO
128-row k-chunks at 0.5 cycles/row - measured 215ns per [128,512]
DoubleRow matmul vs 259ns per half-the-work bf16 matmul.  Chunks are
processed column-pair-major so slab j is first needed ~j/8 of the way
through the GEMM, hiding the HBM stream behind compute (the bf16
baseline was DMA-bound: 5.3MB/core at ~95GB/s; fp8 needs 2.2MB).

The hardest-negative reduction is split across ACT and DVE (DVE may
read only ONE PSUM operand per instruction, so it ingests PSUM at just
1 elem/cycle; TENSOR_REDUCE never engages a packed fast mode on HW,
but fp16 TENSOR_TENSOR runs 2x): 8 of 16 PSUM chunks get a direct DVE
tensor_reduce(max of P); m-tiles 0 and 1 instead get an ACT pass
e = 2 - P/32 (monotone decreasing, = dist^2 + 4S) written as fp16 to
SBUF, folded by per-m-tile fp16 TT-min chains on DVE at 2 elem/cycle
with one 1x final reduce each.  The label-sort windows then read their
positive dist^2 directly off the fp16 e values (relu(e - 4)) where
available.  Host combines both forms, plus the windowed positive sums,
into the loss.  Engine loads balance at ~15.5us each (PE / ACT / DVE)
with the 2.2MB HBM stream fully hidden; measured wall time varies with
the chip's DVFS throttle state.
"""

import sys

if "/opt/trn_rl_repo" not in sys.path:
    sys.path.insert(0, "/opt/trn_rl_repo")

import ml_dtypes
import numpy as np

import concourse.bass as bass
import concourse.bacc as bacc
import concourse.mybir as mybir
import concourse.tile as tile
from concourse.bass_utils import run_bass_kernel_spmd

F32 = mybir.dt.float32
F16 = mybir.dt.float16
FP8 = mybir.dt.float8e4
NP_FP8 = ml_dtypes.float8_e4m3
ALU = mybir.AluOpType
ACTF = mybir.ActivationFunctionType
AXX = mybir.AxisListType.X
PERF = mybir.MatmulPerfMode.DoubleRow

B = 4096
D = 512
C = 64
NCORES = 8
SHARD = B // NCORES          # 512 rows per core
MT = 4                       # m-tiles per core
NJ = 8                       # column slabs of 512
KC = 4                       # data k-chunks of 128 (contracted in 2 pairs)
SCALE = 8.0                  # fp8 input scale; P = 64*ghat - 128*S
# (c, m) psum chunks routed through the ACT e-pass (e = 2 - P/32, fp16
# SBUF) instead of a direct DVE max; per-m TT-min chains on DVE then
# fold the fp16 e-chunks at 2 elem/cycle
ACT_MAX_CHUNKS = {
    (0, 0), (1, 0), (2, 0), (3, 0),
    (0, 1), (1, 1), (2, 1), (3, 1),
}

MARGIN = 1.0


def _plan(maxc):
    """Label-sorted geometry, in LOCAL (rotated) column coords.

    Window of m-tile m = all columns that can share a class with its
    rows: [128m - (maxc-1), 128m + 127 + maxc).  Returns the ACT
    positive-pass segments per 1024-col psum chunk and the (m, slab)
    pairs needing a one-hot matmul.
    """
    wins = []
    for m in range(MT):
        lo = 128 * m - (maxc - 1)
        hi = 128 * m + 128 + (maxc - 1)
        w = min(hi - lo, B)
        ivs = []
        lo %= B
        while w > 0:
            take = min(w, B - lo)
            ivs.append((lo, lo + take))
            lo = 0
            w -= take
        wins.append(ivs)

    segs = []    # (m, c, lo_in_chunk, hi_in_chunk)
    oh = set()   # (m, local slab)
    for m, ivs in enumerate(wins):
        for (a, b) in ivs:
            for c in range(4):
                clo, chi = 1024 * c, 1024 * (c + 1)
                s_lo, s_hi = max(a, clo), min(b, chi)
                if s_lo < s_hi:
                    segs.append((m, c, s_lo - clo, s_hi - clo))
            for s in range(NJ):
                if max(a, 512 * s) < min(b, 512 * (s + 1)):
                    oh.add((m, s))
    oh_slabs = sorted({s for (_, s) in oh})
    return segs, oh, oh_slabs


def _build_nc(maxc):
    segs, oh, oh_slabs = _plan(maxc)
    noh = len(oh_slabs)
    oh_idx = {s: t for t, s in enumerate(oh_slabs)}
    nstat = 16 + len(segs) + 2

    nc = bacc.Bacc(
        "TRN2",
        target_bir_lowering=False,
        debug=False,
        enable_asserts=False,
        num_devices=NCORES,
    )
    atp0 = nc.dram_tensor("atp0", [128, KC, 512], FP8, kind="ExternalInput")
    atp1 = nc.dram_tensor("atp1", [128, KC, 512], FP8, kind="ExternalInput")
    atp23 = nc.dram_tensor("atp23", [128, 2, KC, 512], FP8, kind="ExternalInput")
    atp4567 = nc.dram_tensor(
        "atp4567", [128, 2, 2, KC, 512], FP8, kind="ExternalInput"
    )
    ylr = nc.dram_tensor("ylr", [32, 1 + noh, 2, 512], FP8, kind="ExternalInput")
    stats_d = nc.dram_tensor("stats", [128, nstat], F32, kind="ExternalOutput")

    with tile.TileContext(nc) as tc:
        with (
            tc.tile_pool(name="slabs", bufs=1) as slabs,
            tc.tile_pool(name="psum", bufs=4, space=bass.MemorySpace.PSUM) as psum,
            tc.tile_pool(name="scr", bufs=1) as scr,
            tc.tile_pool(name="esb", bufs=10) as esbp,
            tc.tile_pool(name="chn", bufs=8) as chnp,
            tc.tile_pool(name="stat", bufs=1) as stat,
        ):
            # --- SBUF tiles -------------------------------------------------
            s0 = slabs.tile([128, KC, 512], FP8, name="s0", tag="s0")
            s1 = slabs.tile([128, KC, 512], FP8, name="s1", tag="s1")
            s23 = slabs.tile([128, 2, KC, 512], FP8, name="s23", tag="s23")
            s47 = slabs.tile([128, 2, 2, KC, 512], FP8, name="s47", tag="s47")
            ylrt = stat.tile([32, 1 + noh, 2, 512], FP8, name="ylrt", tag="ylrt")
            parts = stat.tile([128, nstat], F32, name="parts", tag="parts")
            # ACT window scratch
            wt = scr.tile([128, 1024], F32, name="wt", tag="wt")
            dsc = scr.tile([128, 1024], F32, name="dsc", tag="dsc")

            # --- DMA issue.  The scalar (Activation) HWDGE queue measures
            # ~2x the sync queue's bandwidth and throughput scales with the
            # per-partition contiguous run, so tensors are merged into large
            # rows: early pieces stream on scalar, the late 1MB on sync.
            nc.sync.dma_start(ylrt[:], ylr.ap())
            nc.sync.dma_start(s0[:], atp0.ap())
            nc.sync.dma_start(s1[:], atp1.ap())
            nc.sync.dma_start(s23[:], atp23.ap())
            nc.scalar.dma_start(s47[:], atp4567.ap())

            # --- constants & warm-up ---------------------------------------
            bias_c = {}
            for bname, bval in [("m2", -2.0), ("p2", 2.0), ("m4", -4.0), ("z", 0.0)]:
                bt = stat.tile([128, 1], F32, name=f"bc_{bname}", tag=f"bc_{bname}")
                nc.gpsimd.memset(bt[:], bval)
                bias_c[bname] = bt

            # Sqrt first: pulls in the one table set that serves both
            # Sqrt and Relu, so only a single ACT_TABLE_LOAD is paid
            warm = stat.tile([128, 1], F32, name="warm", tag="warm")
            nc.scalar.activation(warm[:], bias_c["z"][:], ACTF.Sqrt,
                                 bias=bias_c["z"][:])
            nc.scalar.activation(warm[:], warm[:], ACTF.Relu,
                                 bias=bias_c["z"][:])

            # PE warm-up: dummy matmuls open the HAM clock gate / p-state
            # ramp while the first slab DMA is in flight
            warm_w = stat.tile([128, 2, 128], FP8, name="warm_w", tag="warm_w")
            warm_x = stat.tile([128, 2, 512], FP8, name="warm_x", tag="warm_x")
            nc.gpsimd.memset(warm_w[:], 0.0)
            nc.gpsimd.memset(warm_x[:], 0.0)
            wpt = psum.tile([128, 512], F32, name="wpt", tag="pt")
            for _ in range(2):
                nc.tensor.matmul(
                    wpt[:], warm_w[:], warm_x[:], start=True, stop=True,
                    perf_mode=PERF,
                )

            # --- main loop: column-pair-major over (chunk, m-tile) ----------
            def rhs_ap(s, kk):
                if s < 2:
                    return (s0, s1)[s][:, 2 * kk : 2 * kk + 2, :]
                if s < 4:
                    return s23[:, s - 2, 2 * kk : 2 * kk + 2, :]
                return s47[:, (s - 4) // 2, (s - 4) % 2, 2 * kk : 2 * kk + 2, :]

            segcol = {}
            for i, (m, c, lo, hi) in enumerate(segs):
                segcol[(m, c, lo, hi)] = 16 + i

            chain = [None] * MT
            last_act_c = {}
            for (c, m) in ACT_MAX_CHUNKS:
                last_act_c[m] = max(last_act_c.get(m, -1), c)

            def emit_region(pt, c, m, sj):
                s = 2 * c + sj
                for kk in range(2):
                    last = kk == 1 and (m, s) not in oh
                    nc.tensor.matmul(
                        pt[:, sj * 512 : (sj + 1) * 512],
                        s0[:, 2 * kk : 2 * kk + 2, m * 128 : (m + 1) * 128],
                        rhs_ap(s, kk),
                        start=(kk == 0),
                        stop=last,
                        perf_mode=PERF,
                    )
                if (m, s) in oh:
                    nc.tensor.matmul(
                        pt[:, sj * 512 : (sj + 1) * 512],
                        ylrt[:, 0, :, m * 128 : (m + 1) * 128],
                        ylrt[:, 1 + oh_idx[s], :, :],
                        start=False,
                        stop=True,
                        perf_mode=PERF,
                    )

            c0_pts = {}
            for c in range(4):
                for m in range(MT):
                    if c == 0:
                        # slab-0 halves of all four m-tiles run first: a
                        # ~2.6us PE runway while slab 1 is still streaming
                        if m == 0:
                            for m_ in range(MT):
                                c0_pts[m_] = psum.tile(
                                    [128, 1024], F32, name="pt", tag="pt"
                                )
                                emit_region(c0_pts[m_], 0, m_, 0)
                        pt = c0_pts[m]
                        emit_region(pt, 0, m, 1)
                    else:
                        pt = psum.tile([128, 1024], F32, name="pt", tag="pt")
                        for sj in range(2):
                            emit_region(pt, c, m, sj)
                    # hardest-negative reduction, split by chunk route
                    et = None
                    if (c, m) in ACT_MAX_CHUNKS:
                        # ACT: e = 2 - P/32 = dist^2 + 4S to fp16 SBUF;
                        # DVE folds it into the m-tile's TT-min chain at
                        # 2 elem/cycle, one 1x final reduce per m-tile
                        et = esbp.tile([128, 1024], F16, name="et", tag="et")
                        nc.scalar.activation(
                            et[:], pt[:], ACTF.Relu,
                            bias=bias_c["p2"][:], scale=-1.0 / 32.0,
                        )
                        if chain[m] is None:
                            chain[m] = et
                        else:
                            r = chnp.tile([128, 1024], F16, name="rc", tag="rc")
                            nc.vector.tensor_tensor(
                                r[:], chain[m][:], et[:], ALU.min
                            )
                            chain[m] = r
                        if c == last_act_c[m]:
                            nc.vector.tensor_reduce(
                                parts[:, 4 * m : 4 * m + 1], chain[m][:],
                                axis=AXX, op=ALU.min,
                            )
                    else:
                        # DVE direct: rowmax(P) from PSUM.  The last two
                        # chunks reduce in 512-col halves so the slab-6
                        # half overlaps the slab-7 matmuls and only one
                        # short reduce trails the final matmul.
                        mcol = 4 * m + c
                        if c == 3 and m >= 2:
                            extra = nstat - (4 - m)
                            nc.vector.tensor_reduce(
                                parts[:, mcol : mcol + 1], pt[:, 0:512],
                                axis=AXX, op=ALU.max,
                            )
                            nc.vector.tensor_reduce(
                                parts[:, extra : extra + 1], pt[:, 512:1024],
                                axis=AXX, op=ALU.max,
                            )
                        else:
                            nc.vector.tensor_reduce(
                                parts[:, mcol : mcol + 1], pt[:],
                                axis=AXX, op=ALU.max,
                            )
                    # positive pass: dist = sqrt(relu(-P/32 - 2)) over the
                    # diagonal window (= sqrt(relu(e - 4)) on the e-path);
                    # accum_out emits the row-sum for free
                    for (m_, c_, lo, hi) in segs:
                        if m_ != m or c_ != c:
                            continue
                        w = hi - lo
                        col = segcol[(m_, c_, lo, hi)]
                        if et is not None:
                            nc.scalar.activation(
                                wt[:, :w], et[:, lo:hi], ACTF.Relu,
                                bias=bias_c["m4"][:],
                            )
                        else:
                            nc.scalar.activation(
                                wt[:, :w], pt[:, lo:hi], ACTF.Relu,
                                bias=bias_c["m2"][:], scale=-1.0 / 32.0,
                            )
                        nc.scalar.activation(
                            dsc[:, :w], wt[:, :w], ACTF.Sqrt,
                            bias=bias_c["z"][:],
                            accum_out=parts[:, col : col + 1],
                        )

            nc.scalar.dma_start(stats_d.ap(), parts[:])

    nc.compile()
    return nc, segs, oh_slabs, nstat


_NC_CACHE: dict = {}


def _get_nc(maxc):
    if maxc not in _NC_CACHE:
        _NC_CACHE[maxc] = _build_nc(maxc)
    return _NC_CACHE[maxc]


def _prep_inputs(embeddings: np.ndarray, labels: np.ndarray):
    E = np.asarray(embeddings, dtype=np.float32)
    L = np.asarray(labels).astype(np.int64)
    assert E.shape == (B, D) and L.shape == (B,)

    order = np.argsort(L, kind="stable")
    Ls = L[order]
    nrm = np.maximum(np.linalg.norm(E, axis=1), 1e-12)
    N = (E / nrm[:, None]).astype(np.float32)[order]

    cnt = np.bincount(Ls, minlength=C)
    maxc = int(cnt.max())
    nc, segs, oh_slabs, nstat = _get_nc(maxc)

    X8 = np.ascontiguousarray((SCALE * N).T.astype(NP_FP8))       # [D, B]
    # S[g][p][c][x] = X8[128c + p, 512g + x]
    S = np.ascontiguousarray(
        X8.reshape(KC, 128, NJ, 512).transpose(2, 1, 0, 3)
    )                                                             # [g,p,c,x]
    Y = (Ls[None, :] == np.arange(C, dtype=np.int64)[:, None]).astype(np.float32)

    in_maps = []
    for r in range(NCORES):
        Sr = np.roll(S, -r, axis=0)                               # local j
        rows = slice(SHARD * r, SHARD * (r + 1))
        ylc = np.ascontiguousarray(
            (2 * SCALE * Y[:, rows]).reshape(2, 32, SHARD)
            .transpose(1, 0, 2).astype(NP_FP8)
        )
        yrr = np.stack(
            [
                (-SCALE * Y[:, 512 * ((r + s) % NJ) : 512 * ((r + s) % NJ) + 512])
                .reshape(2, 32, 512)
                for s in oh_slabs
            ]
        )                                                         # [t,h,p,x]
        yrr = np.ascontiguousarray(yrr.transpose(2, 0, 1, 3).astype(NP_FP8))
        in_maps.append(
            {
                "atp0": np.ascontiguousarray(Sr[0]),
                "atp1": np.ascontiguousarray(Sr[1]),
                "atp23": np.ascontiguousarray(Sr[2:4].transpose(1, 0, 2, 3)),
                "atp4567": np.ascontiguousarray(
                    Sr[4:8].reshape(2, 2, 128, KC, 512).transpose(2, 0, 1, 3, 4)
                ),
                "ylr": np.ascontiguousarray(
                    np.concatenate([ylc[:, None, :, :], yrr], axis=1)
                ),
            }
        )

    pos_cnt = cnt[Ls] - 1
    neg_cnt = B - cnt[Ls]
    invc = (1.0 / np.maximum(pos_cnt, 1)).astype(np.float32)
    valid = ((pos_cnt > 0) & (neg_cnt > 0)).astype(np.float32)
    return nc, segs, nstat, in_maps, (invc, valid)


def _finish(results, segs, nstat, aux):
    invc, valid = aux
    pos_sum = np.empty(B, dtype=np.float32)
    neg2 = np.empty(B, dtype=np.float32)
    for r in range(NCORES):
        st = np.asarray(results[r]["stats"])                      # [128, nstat]
        grid = st[:, :16].reshape(128, MT, 4)
        act_ms = {m for (_, m) in ACT_MAX_CHUNKS}
        n2 = np.full((128, MT), np.inf, dtype=np.float32)
        for m in range(MT):
            if m in act_ms:
                # the m-tile's TT-min chain result lands in col 4m+0
                n2[:, m] = np.minimum(n2[:, m], grid[:, m, 0])
            for c in range(4):
                if (c, m) in ACT_MAX_CHUNKS or (c == 0 and m in act_ms):
                    continue
                n2[:, m] = np.minimum(n2[:, m], 2.0 - grid[:, m, c] / 32.0)
        for m in (2, 3):
            extra = nstat - (4 - m)
            n2[:, m] = np.minimum(n2[:, m], 2.0 - st[:, extra] / 32.0)
        ps = np.zeros((128, MT), dtype=np.float32)
        for i, (m, c, lo, hi) in enumerate(segs):
            ps[:, m] += st[:, 16 + i]
        rows = slice(SHARD * r, SHARD * (r + 1))
        pos_sum[rows] = ps.T.reshape(SHARD)
        neg2[rows] = n2.T.reshape(SHARD)
    pos_stat = pos_sum * invc
    neg_stat = np.sqrt(np.maximum(neg2, 0.0), dtype=np.float32)
    per_row = np.maximum(pos_stat - neg_stat + MARGIN, 0.0) * valid
    n_valid = float(valid.sum())
    total = float(per_row.sum(dtype=np.float32))
    out = total / max(n_valid, 1.0) if n_valid > 0 else 0.0
    return np.array(out, dtype=np.float32)


def kernel(embeddings, labels, _run_kwargs=None):
    nc, segs, nstat, in_maps, aux = _prep_inputs(embeddings, labels)
    res = run_bass_kernel_spmd(
        nc, in_maps, core_ids=list(range(NCORES)), **(_run_kwargs or {})
    )
    out = _finish(res.results, segs, nstat, aux)
    if _run_kwargs:
        return out, res
    return out


# revision 24
# speedup vs baseline: 1.0573x; 1.0476x over previous
"""HardTripletLoss on 8 Trainium2 NeuronCores (Bass/Tile), fp8 edition.

Math
----
reference: emb = l2_normalize(embeddings); dist = cdist(emb, emb);
  pos_stat[i] = mean_{j: same class, j!=i} dist[i,j]
  neg_stat[i] = min_{j: diff class} dist[i,j]
  loss = mean over valid rows of relu(pos_stat - neg_stat + 1)

For unit vectors dist^2 = 2 - 2*ghat with ghat = N @ N.T.  We quantize
X = 8*N to fp8e4m3 (measured end-to-end rel err ~5e-4) and fold the
class mask into the GEMM: P = X@X.T - 128*S = 64*ghat - 128*S, with the
-128*S term contributed by a small one-hot block (lhsT 16*Y, rhs -8*Y,
Y = onehot(labels)).  Then per row:
  positive dists = sqrt(relu(-P/32 - 2))     (diff-class and diagonal -> 0)
  hardest negative^2 = min over row of e, e = 2 - P/32 = dist^2 + 4*S

Host-side trick: rows are SORTED BY LABEL, so each row's same-class
columns live in a narrow diagonal band.  The sqrt/accumulate positive
pass (ACT engine) then only touches a ~(128+2*maxc)-wide column window
per 128-row m-tile instead of all 4096 columns, and the one-hot GEMM
blocks are only emitted for the 2-3 slabs that windows touch.  Sorting
is a symmetric permutation of dist: per-row stats permute with it and
the final mean is unchanged.

Sharding: rows split 512/core (data parallel).  Every core holds all
4096 columns as 8 fp8 slabs of 512 cols; slab order is rotated per core
so slab 0 contains its own shard columns (the matmul stationary
operand) and the label-sorted diagonal windows sit at a core-invariant
position - one SPMD program for all 8 cores.

GEMM runs in fp8 DoubleRow perf mode: each matmul contracts TWO
128-row k-chunks at 0.5 cycles/row - measured 215ns per [128,512]
DoubleRow matmul vs 259ns per half-the-work bf16 matmul.  Chunks are
processed column-pair-major so slab j is first needed ~j/8 of the way
through the GEMM, hiding the HBM stream behind compute (the bf16
baseline was DMA-bound: 5.3MB/core at ~95GB/s; fp8 needs 2.2MB).

The hardest-negative reduction is split across ACT and DVE (DVE may
read only ONE PSUM operand per instruction, so it ingests PSUM at just
1 elem/cycle; TENSOR_REDUCE never engages a packed fast mode on HW,
but fp16 TENSOR_TENSOR runs 2x): 8 of 16 PSUM chunks get a direct DVE
tensor_reduce(max of P); m-tiles 0 and 1 instead get an ACT pass
e = 2 - P/32 (monotone decreasing, = dist^2 + 4S) written as fp16 to
SBUF, folded by per-m-tile fp16 TT-min chains on DVE at 2 elem/cycle
with one 1x final reduce each.  The label-sort windows then read their
positive dist^2 directly off the fp16 e values (relu(e - 4)) where
available.  Host combines both forms, plus the windowed positive sums,
into the loss.  Engine loads balance at ~15.5us each (PE / ACT / DVE)
with the 2.2MB HBM stream fully hidden; measured wall time varies with
the chip's DVFS throttle state.
"""

import sys

if "/opt/trn_rl_repo" not in sys.path:
    sys.path.insert(0, "/opt/trn_rl_repo")

import ml_dtypes
import numpy as np

import concourse.bass as bass
import concourse.bacc as bacc
import concourse.mybir as mybir
import concourse.tile as tile
from concourse.bass_utils import run_bass_kernel_spmd

F32 = mybir.dt.float32
F16 = mybir.dt.float16
FP8 = mybir.dt.float8e4
NP_FP8 = ml_dtypes.float8_e4m3
ALU = mybir.AluOpType
ACTF = mybir.ActivationFunctionType
AXX = mybir.AxisListType.X
PERF = mybir.MatmulPerfMode.DoubleRow

B = 4096
D = 512
C = 64
NCORES = 8
SHARD = B // NCORES          # 512 rows per core
MT = 4                       # m-tiles per core
NJ = 8                       # column slabs of 512
KC = 4                       # data k-chunks of 128 (contracted in 2 pairs)
SCALE = 8.0                  # fp8 input scale; P = 64*ghat - 128*S
# (c, m) psum chunks routed through the ACT e-pass (e = 2 - P/32, fp16
# SBUF) instead of a direct DVE max; per-m TT-min chains on DVE then
# fold the fp16 e-chunks at 2 elem/cycle
ACT_MAX_CHUNKS = {
    (0, 0), (1, 0), (2, 0),
    (0, 1), (1, 1), (2, 1),
}

MARGIN = 1.0


def _plan(maxc):
    """Label-sorted geometry, in LOCAL (rotated) column coords.

    Window of m-tile m = all columns that can share a class with its
    rows: [128m - (maxc-1), 128m + 127 + maxc).  Returns the ACT
    positive-pass segments per 1024-col psum chunk and the (m, slab)
    pairs needing a one-hot matmul.
    """
    wins = []
    for m in range(MT):
        lo = 128 * m - (maxc - 1)
        hi = 128 * m + 128 + (maxc - 1)
        w = min(hi - lo, B)
        ivs = []
        lo %= B
        while w > 0:
            take = min(w, B - lo)
            ivs.append((lo, lo + take))
            lo = 0
            w -= take
        wins.append(ivs)

    segs = []    # (m, c, lo_in_chunk, hi_in_chunk)
    oh = set()   # (m, local slab)
    for m, ivs in enumerate(wins):
        for (a, b) in ivs:
            for c in range(4):
                clo, chi = 1024 * c, 1024 * (c + 1)
                s_lo, s_hi = max(a, clo), min(b, chi)
                if s_lo < s_hi:
                    segs.append((m, c, s_lo - clo, s_hi - clo))
            for s in range(NJ):
                if max(a, 512 * s) < min(b, 512 * (s + 1)):
                    oh.add((m, s))
    oh_slabs = sorted({s for (_, s) in oh})
    return segs, oh, oh_slabs


def _build_nc(maxc):
    segs, oh, oh_slabs = _plan(maxc)
    noh = len(oh_slabs)
    oh_idx = {s: t for t, s in enumerate(oh_slabs)}
    nstat = 16 + len(segs) + 4

    nc = bacc.Bacc(
        "TRN2",
        target_bir_lowering=False,
        debug=False,
        enable_asserts=False,
        num_devices=NCORES,
    )
    atp0 = nc.dram_tensor("atp0", [128, KC, 512], FP8, kind="ExternalInput")
    atp1 = nc.dram_tensor("atp1", [128, KC, 512], FP8, kind="ExternalInput")
    atp23 = nc.dram_tensor("atp23", [128, 2, KC, 512], FP8, kind="ExternalInput")
    atp4567 = nc.dram_tensor(
        "atp4567", [128, 2, 2, KC, 512], FP8, kind="ExternalInput"
    )
    ylr = nc.dram_tensor("ylr", [32, 1 + noh, 2, 512], FP8, kind="ExternalInput")
    stats_d = nc.dram_tensor("stats", [128, nstat], F32, kind="ExternalOutput")

    with tile.TileContext(nc) as tc:
        with (
            tc.tile_pool(name="slabs", bufs=1) as slabs,
            tc.tile_pool(name="psum", bufs=4, space=bass.MemorySpace.PSUM) as psum,
            tc.tile_pool(name="scr", bufs=1) as scr,
            tc.tile_pool(name="esb", bufs=10) as esbp,
            tc.tile_pool(name="chn", bufs=8) as chnp,
            tc.tile_pool(name="stat", bufs=1) as stat,
        ):
            # --- SBUF tiles -------------------------------------------------
            s0 = slabs.tile([128, KC, 512], FP8, name="s0", tag="s0")
            s1 = slabs.tile([128, KC, 512], FP8, name="s1", tag="s1")
            s23 = slabs.tile([128, 2, KC, 512], FP8, name="s23", tag="s23")
            s47 = slabs.tile([128, 2, 2, KC, 512], FP8, name="s47", tag="s47")
            ylrt = stat.tile([32, 1 + noh, 2, 512], FP8, name="ylrt", tag="ylrt")
            parts = stat.tile([128, nstat], F32, name="parts", tag="parts")
            # ACT window scratch
            wt = scr.tile([128, 1024], F32, name="wt", tag="wt")
            dsc = scr.tile([128, 1024], F32, name="dsc", tag="dsc")

            # --- DMA issue.  The scalar (Activation) HWDGE queue measures
            # ~2x the sync queue's bandwidth and throughput scales with the
            # per-partition contiguous run, so tensors are merged into large
            # rows: early pieces stream on scalar, the late 1MB on sync.
            nc.sync.dma_start(ylrt[:], ylr.ap())
            nc.sync.dma_start(s0[:], atp0.ap())
            nc.sync.dma_start(s1[:], atp1.ap())
            nc.sync.dma_start(s23[:], atp23.ap())
            nc.scalar.dma_start(s47[:], atp4567.ap())

            # --- constants & warm-up ---------------------------------------
            bias_c = {}
            for bname, bval in [("m2", -2.0), ("p2", 2.0), ("m4", -4.0), ("z", 0.0)]:
                bt = stat.tile([128, 1], F32, name=f"bc_{bname}", tag=f"bc_{bname}")
                nc.gpsimd.memset(bt[:], bval)
                bias_c[bname] = bt

            # Sqrt first: pulls in the one table set that serves both
            # Sqrt and Relu, so only a single ACT_TABLE_LOAD is paid
            warm = stat.tile([128, 1], F32, name="warm", tag="warm")
            nc.scalar.activation(warm[:], bias_c["z"][:], ACTF.Sqrt,
                                 bias=bias_c["z"][:])
            nc.scalar.activation(warm[:], warm[:], ACTF.Relu,
                                 bias=bias_c["z"][:])

            # PE warm-up: dummy matmuls open the HAM clock gate / p-state
            # ramp while the first slab DMA is in flight
            warm_w = stat.tile([128, 2, 128], FP8, name="warm_w", tag="warm_w")
            warm_x = stat.tile([128, 2, 512], FP8, name="warm_x", tag="warm_x")
            nc.gpsimd.memset(warm_w[:], 0.0)
            nc.gpsimd.memset(warm_x[:], 0.0)
            wpt = psum.tile([128, 512], F32, name="wpt", tag="pt")
            for _ in range(2):
                nc.tensor.matmul(
                    wpt[:], warm_w[:], warm_x[:], start=True, stop=True,
                    perf_mode=PERF,
                )

            # --- main loop: column-pair-major over (chunk, m-tile) ----------
            def rhs_ap(s, kk):
                if s < 2:
                    return (s0, s1)[s][:, 2 * kk : 2 * kk + 2, :]
                if s < 4:
                    return s23[:, s - 2, 2 * kk : 2 * kk + 2, :]
                return s47[:, (s - 4) // 2, (s - 4) % 2, 2 * kk : 2 * kk + 2, :]

            segcol = {}
            for i, (m, c, lo, hi) in enumerate(segs):
                segcol[(m, c, lo, hi)] = 16 + i

            chain = [None] * MT
            last_act_c = {}
            for (c, m) in ACT_MAX_CHUNKS:
                last_act_c[m] = max(last_act_c.get(m, -1), c)

            def emit_region(pt, c, m, sj):
                s = 2 * c + sj
                for kk in range(2):
                    last = kk == 1 and (m, s) not in oh
                    nc.tensor.matmul(
                        pt[:, sj * 512 : (sj + 1) * 512],
                        s0[:, 2 * kk : 2 * kk + 2, m * 128 : (m + 1) * 128],
                        rhs_ap(s, kk),
                        start=(kk == 0),
                        stop=last,
                        perf_mode=PERF,
                    )
                if (m, s) in oh:
                    nc.tensor.matmul(
                        pt[:, sj * 512 : (sj + 1) * 512],
                        ylrt[:, 0, :, m * 128 : (m + 1) * 128],
                        ylrt[:, 1 + oh_idx[s], :, :],
                        start=False,
                        stop=True,
                        perf_mode=PERF,
                    )

            c0_pts = {}
            for c in range(4):
                for m in range(MT):
                    if c == 0:
                        # slab-0 halves of all four m-tiles run first: a
                        # ~2.6us PE runway while slab 1 is still streaming
                        if m == 0:
                            for m_ in range(MT):
                                c0_pts[m_] = psum.tile(
                                    [128, 1024], F32, name="pt", tag="pt"
                                )
                                emit_region(c0_pts[m_], 0, m_, 0)
                        pt = c0_pts[m]
                        emit_region(pt, 0, m, 1)
                    else:
                        pt = psum.tile([128, 1024], F32, name="pt", tag="pt")
                        for sj in range(2):
                            emit_region(pt, c, m, sj)
                    # hardest-negative reduction, split by chunk route
                    et = None
                    if (c, m) in ACT_MAX_CHUNKS:
                        # ACT: e = 2 - P/32 = dist^2 + 4S to fp16 SBUF;
                        # DVE folds it into the m-tile's TT-min chain at
                        # 2 elem/cycle, one 1x final reduce per m-tile
                        et = esbp.tile([128, 1024], F16, name="et", tag="et")
                        nc.scalar.activation(
                            et[:], pt[:], ACTF.Relu,
                            bias=bias_c["p2"][:], scale=-1.0 / 32.0,
                        )
                        if chain[m] is None:
                            chain[m] = et
                        else:
                            r = chnp.tile([128, 1024], F16, name="rc", tag="rc")
                            nc.vector.tensor_tensor(
                                r[:], chain[m][:], et[:], ALU.min
                            )
                            chain[m] = r
                        if c == last_act_c[m]:
                            nc.vector.tensor_reduce(
                                parts[:, 4 * m : 4 * m + 1], chain[m][:],
                                axis=AXX, op=ALU.min,
                            )
                    else:
                        # DVE direct: rowmax(P) from PSUM.  The last two
                        # chunks reduce in 512-col halves so the slab-6
                        # half overlaps the slab-7 matmuls and only one
                        # short reduce trails the final matmul.
                        mcol = 4 * m + c
                        if c == 3:
                            extra = nstat - 4 + m
                            nc.vector.tensor_reduce(
                                parts[:, mcol : mcol + 1], pt[:, 0:512],
                                axis=AXX, op=ALU.max,
                            )
                            nc.vector.tensor_reduce(
                                parts[:, extra : extra + 1], pt[:, 512:1024],
                                axis=AXX, op=ALU.max,
                            )
                        else:
                            nc.vector.tensor_reduce(
                                parts[:, mcol : mcol + 1], pt[:],
                                axis=AXX, op=ALU.max,
                            )
                    # positive pass: dist = sqrt(relu(-P/32 - 2)) over the
                    # diagonal window (= sqrt(relu(e - 4)) on the e-path);
                    # accum_out emits the row-sum for free
                    for (m_, c_, lo, hi) in segs:
                        if m_ != m or c_ != c:
                            continue
                        w = hi - lo
                        col = segcol[(m_, c_, lo, hi)]
                        if et is not None:
                            nc.scalar.activation(
                                wt[:, :w], et[:, lo:hi], ACTF.Relu,
                                bias=bias_c["m4"][:],
                            )
                        else:
                            nc.scalar.activation(
                                wt[:, :w], pt[:, lo:hi], ACTF.Relu,
                                bias=bias_c["m2"][:], scale=-1.0 / 32.0,
                            )
                        nc.scalar.activation(
                            dsc[:, :w], wt[:, :w], ACTF.Sqrt,
                            bias=bias_c["z"][:],
                            accum_out=parts[:, col : col + 1],
                        )

            nc.scalar.dma_start(stats_d.ap(), parts[:])

    nc.compile()
    return nc, segs, oh_slabs, nstat


_NC_CACHE: dict = {}


def _get_nc(maxc):
    if maxc not in _NC_CACHE:
        _NC_CACHE[maxc] = _build_nc(maxc)
    return _NC_CACHE[maxc]


def _prep_inputs(embeddings: np.ndarray, labels: np.ndarray):
    E = np.asarray(embeddings, dtype=np.float32)
    L = np.asarray(labels).astype(np.int64)
    assert E.shape == (B, D) and L.shape == (B,)

    order = np.argsort(L, kind="stable")
    Ls = L[order]
    nrm = np.maximum(np.linalg.norm(E, axis=1), 1e-12)
    N = (E / nrm[:, None]).astype(np.float32)[order]

    cnt = np.bincount(Ls, minlength=C)
    maxc = int(cnt.max())
    nc, segs, oh_slabs, nstat = _get_nc(maxc)

    X8 = np.ascontiguousarray((SCALE * N).T.astype(NP_FP8))       # [D, B]
    # S[g][p][c][x] = X8[128c + p, 512g + x]
    S = np.ascontiguousarray(
        X8.reshape(KC, 128, NJ, 512).transpose(2, 1, 0, 3)
    )                                                             # [g,p,c,x]
    Y = (Ls[None, :] == np.arange(C, dtype=np.int64)[:, None]).astype(np.float32)

    in_maps = []
    for r in range(NCORES):
        Sr = np.roll(S, -r, axis=0)                               # local j
        rows = slice(SHARD * r, SHARD * (r + 1))
        ylc = np.ascontiguousarray(
            (2 * SCALE * Y[:, rows]).reshape(2, 32, SHARD)
            .transpose(1, 0, 2).astype(NP_FP8)
        )
        yrr = np.stack(
            [
                (-SCALE * Y[:, 512 * ((r + s) % NJ) : 512 * ((r + s) % NJ) + 512])
                .reshape(2, 32, 512)
                for s in oh_slabs
            ]
        )                                                         # [t,h,p,x]
        yrr = np.ascontiguousarray(yrr.transpose(2, 0, 1, 3).astype(NP_FP8))
        in_maps.append(
            {
                "atp0": np.ascontiguousarray(Sr[0]),
                "atp1": np.ascontiguousarray(Sr[1]),
                "atp23": np.ascontiguousarray(Sr[2:4].transpose(1, 0, 2, 3)),
                "atp4567": np.ascontiguousarray(
                    Sr[4:8].reshape(2, 2, 128, KC, 512).transpose(2, 0, 1, 3, 4)
                ),
                "ylr": np.ascontiguousarray(
                    np.concatenate([ylc[:, None, :, :], yrr], axis=1)
                ),
            }
        )

    pos_cnt = cnt[Ls] - 1
    neg_cnt = B - cnt[Ls]
    invc = (1.0 / np.maximum(pos_cnt, 1)).astype(np.float32)
    valid = ((pos_cnt > 0) & (neg_cnt > 0)).astype(np.float32)
    return nc, segs, nstat, in_maps, (invc, valid)


def _finish(results, segs, nstat, aux):
    invc, valid = aux
    pos_sum = np.empty(B, dtype=np.float32)
    neg2 = np.empty(B, dtype=np.float32)
    for r in range(NCORES):
        st = np.asarray(results[r]["stats"])                      # [128, nstat]
        grid = st[:, :16].reshape(128, MT, 4)
        act_ms = {m for (_, m) in ACT_MAX_CHUNKS}
        n2 = np.full((128, MT), np.inf, dtype=np.float32)
        for m in range(MT):
            if m in act_ms:
                # the m-tile's TT-min chain result lands in col 4m+0
                n2[:, m] = np.minimum(n2[:, m], grid[:, m, 0])
            for c in range(4):
                if (c, m) in ACT_MAX_CHUNKS or (c == 0 and m in act_ms):
                    continue
                n2[:, m] = np.minimum(n2[:, m], 2.0 - grid[:, m, c] / 32.0)
        for m in range(MT):
            extra = nstat - 4 + m
            n2[:, m] = np.minimum(n2[:, m], 2.0 - st[:, extra] / 32.0)
        ps = np.zeros((128, MT), dtype=np.float32)
        for i, (m, c, lo, hi) in enumerate(segs):
            ps[:, m] += st[:, 16 + i]
        rows = slice(SHARD * r, SHARD * (r + 1))
        pos_sum[rows] = ps.T.reshape(SHARD)
        neg2[rows] = n2.T.reshape(SHARD)
    pos_stat = pos_sum * invc
    neg_stat = np.sqrt(np.maximum(neg2, 0.0), dtype=np.float32)
    per_row = np.maximum(pos_stat - neg_stat + MARGIN, 0.0) * valid
    n_valid = float(valid.sum())
    total = float(per_row.sum(dtype=np.float32))
    out = total / max(n_valid, 1.0) if n_valid > 0 else 0.0
    return np.array(out, dtype=np.float32)


def kernel(embeddings, labels, _run_kwargs=None):
    nc, segs, nstat, in_maps, aux = _prep_inputs(embeddings, labels)
    res = run_bass_kernel_spmd(
        nc, in_maps, core_ids=list(range(NCORES)), **(_run_kwargs or {})
    )
    out = _finish(res.results, segs, nstat, aux)
    if _run_kwargs:
        return out, res
    return out


# revision 25
# speedup vs baseline: 1.2050x; 1.1397x over previous
"""HardTripletLoss on 8 Trainium2 NeuronCores (Bass/Tile), fp8 edition.

Math
----
reference: emb = l2_normalize(embeddings); dist = cdist(emb, emb);
  pos_stat[i] = mean_{j: same class, j!=i} dist[i,j]
  neg_stat[i] = min_{j: diff class} dist[i,j]
  loss = mean over valid rows of relu(pos_stat - neg_stat + 1)

For unit vectors dist^2 = 2 - 2*ghat with ghat = N @ N.T.  We quantize
X = 8*N to fp8e4m3 (measured end-to-end rel err ~5e-4) and fold the
class mask into the GEMM: P = X@X.T - 128*S = 64*ghat - 128*S, with the
-128*S term contributed by a small one-hot block (lhsT 16*Y, rhs -8*Y,
Y = onehot(labels)).  Then per row:
  positive dists = sqrt(relu(-P/32 - 2))     (diff-class and diagonal -> 0)
  hardest negative^2 = min over row of e, e = 2 - P/32 = dist^2 + 4*S

Host-side trick: rows are SORTED BY LABEL, so each row's same-class
columns live in a narrow diagonal band.  The sqrt/accumulate positive
pass (ACT engine) then only touches a ~(128+2*maxc)-wide column window
per 128-row m-tile instead of all 4096 columns, and the one-hot GEMM
blocks are only emitted for the 2-3 slabs that windows touch.  Sorting
is a symmetric permutation of dist: per-row stats permute with it and
the final mean is unchanged.

Sharding: rows split 512/core (data parallel).  Every core holds all
4096 columns as 8 fp8 slabs of 512 cols; slab order is rotated per core
so slab 0 contains its own shard columns (the matmul stationary
operand) and the label-sorted diagonal windows sit at a core-invariant
position - one SPMD program for all 8 cores.

GEMM runs in fp8 DoubleRow perf mode: each matmul contracts TWO
128-row k-chunks at 0.5 cycles/row - measured 215ns per [128,512]
DoubleRow matmul vs 259ns per half-the-work bf16 matmul.  Chunks are
processed column-pair-major so slab j is first needed ~j/8 of the way
through the GEMM, hiding the HBM stream behind compute (the bf16
baseline was DMA-bound: 5.3MB/core at ~95GB/s; fp8 needs 2.2MB).

The hardest-negative reduction is split across ACT and DVE (DVE may
read only ONE PSUM operand per instruction, so it ingests PSUM at just
1 elem/cycle; TENSOR_REDUCE never engages a packed fast mode on HW,
but fp16 TENSOR_TENSOR runs 2x): 8 of 16 PSUM chunks get a direct DVE
tensor_reduce(max of P); m-tiles 0 and 1 instead get an ACT pass
e = 2 - P/32 (monotone decreasing, = dist^2 + 4S) written as fp16 to
SBUF, folded by per-m-tile fp16 TT-min chains on DVE at 2 elem/cycle
with one 1x final reduce each.  The label-sort windows then read their
positive dist^2 directly off the fp16 e values (relu(e - 4)) where
available.  Host combines both forms, plus the windowed positive sums,
into the loss.  Engine loads balance at ~15.5us each (PE / ACT / DVE)
with the 2.2MB HBM stream fully hidden; measured wall time varies with
the chip's DVFS throttle state.
"""

import sys

if "/opt/trn_rl_repo" not in sys.path:
    sys.path.insert(0, "/opt/trn_rl_repo")

import ml_dtypes
import numpy as np

import concourse.bass as bass
import concourse.bacc as bacc
import concourse.mybir as mybir
import concourse.tile as tile
from concourse.bass_utils import run_bass_kernel_spmd

F32 = mybir.dt.float32
F16 = mybir.dt.float16
FP8 = mybir.dt.float8e4
NP_FP8 = ml_dtypes.float8_e4m3
ALU = mybir.AluOpType
ACTF = mybir.ActivationFunctionType
AXX = mybir.AxisListType.X
PERF = mybir.MatmulPerfMode.DoubleRow

B = 4096
D = 512
C = 64
NCORES = 8
SHARD = B // NCORES          # 512 rows per core
MT = 4                       # m-tiles per core
NJ = 8                       # column slabs of 512
KC = 4                       # data k-chunks of 128 (contracted in 2 pairs)
SCALE = 8.0                  # fp8 input scale; P = 64*ghat - 128*S
# (c, m) psum chunks routed through the ACT e-pass (e = 2 - P/32, fp16
# SBUF) instead of a direct DVE max; per-m TT-min chains on DVE then
# fold the fp16 e-chunks at 2 elem/cycle
ACT_MAX_CHUNKS = {
    (0, 0), (1, 0), (2, 0),
    (0, 1), (1, 1), (2, 1),
}

MARGIN = 1.0


def _plan(maxc):
    """Label-sorted geometry, in LOCAL (rotated) column coords.

    Window of m-tile m = all columns that can share a class with its
    rows: [128m - (maxc-1), 128m + 127 + maxc).  Returns the ACT
    positive-pass segments per 1024-col psum chunk and the (m, slab)
    pairs needing a one-hot matmul.
    """
    wins = []
    for m in range(MT):
        lo = 128 * m - (maxc - 1)
        hi = 128 * m + 128 + (maxc - 1)
        w = min(hi - lo, B)
        ivs = []
        lo %= B
        while w > 0:
            take = min(w, B - lo)
            ivs.append((lo, lo + take))
            lo = 0
            w -= take
        wins.append(ivs)

    segs = []    # (m, c, lo_in_chunk, hi_in_chunk)
    oh = set()   # (m, local slab)
    for m, ivs in enumerate(wins):
        for (a, b) in ivs:
            for c in range(4):
                clo, chi = 1024 * c, 1024 * (c + 1)
                s_lo, s_hi = max(a, clo), min(b, chi)
                if s_lo < s_hi:
                    segs.append((m, c, s_lo - clo, s_hi - clo))
            for s in range(NJ):
                if max(a, 512 * s) < min(b, 512 * (s + 1)):
                    oh.add((m, s))
    oh_slabs = sorted({s for (_, s) in oh})
    return segs, oh, oh_slabs


def _build_nc(maxc):
    segs, oh, oh_slabs = _plan(maxc)
    noh = len(oh_slabs)
    oh_idx = {s: t for t, s in enumerate(oh_slabs)}
    nstat = 16 + len(segs) + 4

    nc = bacc.Bacc(
        "TRN2",
        target_bir_lowering=False,
        debug=False,
        enable_asserts=False,
        num_devices=NCORES,
    )
    atp0 = nc.dram_tensor("atp0", [128, KC, 512], FP8, kind="ExternalInput")
    atp1 = nc.dram_tensor("atp1", [128, KC, 512], FP8, kind="ExternalInput")
    atp23 = nc.dram_tensor("atp23", [128, 2, KC, 512], FP8, kind="ExternalInput")
    atp4567 = nc.dram_tensor(
        "atp4567", [128, 2, 2, KC, 512], FP8, kind="ExternalInput"
    )
    ylr = nc.dram_tensor("ylr", [32, 1 + noh, 2, 512], FP8, kind="ExternalInput")
    stats_d = nc.dram_tensor("stats", [128, nstat], F32, kind="ExternalOutput")

    with tile.TileContext(nc) as tc:
        with (
            tc.tile_pool(name="slabs", bufs=1) as slabs,
            tc.tile_pool(name="psum", bufs=4, space=bass.MemorySpace.PSUM) as psum,
            tc.tile_pool(name="scr", bufs=1) as scr,
            tc.tile_pool(name="esb", bufs=10) as esbp,
            tc.tile_pool(name="chn", bufs=8) as chnp,
            tc.tile_pool(name="stat", bufs=1) as stat,
        ):
            # --- SBUF tiles -------------------------------------------------
            s0 = slabs.tile([128, KC, 512], FP8, name="s0", tag="s0")
            s1 = slabs.tile([128, KC, 512], FP8, name="s1", tag="s1")
            s23 = slabs.tile([128, 2, KC, 512], FP8, name="s23", tag="s23")
            s47 = slabs.tile([128, 2, 2, KC, 512], FP8, name="s47", tag="s47")
            ylrt = stat.tile([32, 1 + noh, 2, 512], FP8, name="ylrt", tag="ylrt")
            parts = stat.tile([128, nstat], F32, name="parts", tag="parts")
            # ACT window scratch
            wt = scr.tile([128, 1024], F32, name="wt", tag="wt")
            dsc = scr.tile([128, 1024], F32, name="dsc", tag="dsc")

            # --- DMA issue.  The scalar (Activation) HWDGE queue measures
            # ~2x the sync queue's bandwidth and throughput scales with the
            # per-partition contiguous run, so tensors are merged into large
            # rows: early pieces stream on scalar, the late 1MB on sync.
            nc.sync.dma_start(s0[:], atp0.ap())
            nc.sync.dma_start(ylrt[:], ylr.ap())
            nc.sync.dma_start(s1[:], atp1.ap())
            nc.sync.dma_start(s23[:], atp23.ap())
            nc.scalar.dma_start(s47[:], atp4567.ap())

            # --- constants & warm-up ---------------------------------------
            bias_c = {}
            for bname, bval in [("m2", -2.0), ("p2", 2.0), ("m4", -4.0), ("z", 0.0)]:
                bt = stat.tile([128, 1], F32, name=f"bc_{bname}", tag=f"bc_{bname}")
                nc.gpsimd.memset(bt[:], bval)
                bias_c[bname] = bt

            # Sqrt first: pulls in the one table set that serves both
            # Sqrt and Relu, so only a single ACT_TABLE_LOAD is paid
            warm = stat.tile([128, 1], F32, name="warm", tag="warm")
            nc.scalar.activation(warm[:], bias_c["z"][:], ACTF.Sqrt,
                                 bias=bias_c["z"][:])
            nc.scalar.activation(warm[:], warm[:], ACTF.Relu,
                                 bias=bias_c["z"][:])

            # PE warm-up: dummy matmuls open the HAM clock gate / p-state
            # ramp while the first slab DMA is in flight
            warm_w = stat.tile([128, 2, 128], FP8, name="warm_w", tag="warm_w")
            warm_x = stat.tile([128, 2, 512], FP8, name="warm_x", tag="warm_x")
            nc.gpsimd.memset(warm_w[:], 0.0)
            nc.gpsimd.memset(warm_x[:], 0.0)
            wpt = psum.tile([128, 512], F32, name="wpt", tag="pt")
            for _ in range(2):
                nc.tensor.matmul(
                    wpt[:], warm_w[:], warm_x[:], start=True, stop=True,
                    perf_mode=PERF,
                )

            # --- main loop: column-pair-major over (chunk, m-tile) ----------
            def rhs_ap(s, kk):
                if s < 2:
                    return (s0, s1)[s][:, 2 * kk : 2 * kk + 2, :]
                if s < 4:
                    return s23[:, s - 2, 2 * kk : 2 * kk + 2, :]
                return s47[:, (s - 4) // 2, (s - 4) % 2, 2 * kk : 2 * kk + 2, :]

            segcol = {}
            for i, (m, c, lo, hi) in enumerate(segs):
                segcol[(m, c, lo, hi)] = 16 + i

            chain = [None] * MT
            last_act_c = {}
            for (c, m) in ACT_MAX_CHUNKS:
                last_act_c[m] = max(last_act_c.get(m, -1), c)

            def emit_region_data(pt, c, m, sj):
                s = 2 * c + sj
                for kk in range(2):
                    last = kk == 1 and (m, s) not in oh
                    nc.tensor.matmul(
                        pt[:, sj * 512 : (sj + 1) * 512],
                        s0[:, 2 * kk : 2 * kk + 2, m * 128 : (m + 1) * 128],
                        rhs_ap(s, kk),
                        start=(kk == 0),
                        stop=last,
                        perf_mode=PERF,
                    )

            def emit_region_oh(pt, c, m, sj):
                s = 2 * c + sj
                if (m, s) in oh:
                    nc.tensor.matmul(
                        pt[:, sj * 512 : (sj + 1) * 512],
                        ylrt[:, 0, :, m * 128 : (m + 1) * 128],
                        ylrt[:, 1 + oh_idx[s], :, :],
                        start=False,
                        stop=True,
                        perf_mode=PERF,
                    )

            def emit_region(pt, c, m, sj):
                emit_region_data(pt, c, m, sj)
                emit_region_oh(pt, c, m, sj)

            c0_pts = {}
            for c in range(4):
                for m in range(MT):
                    if c == 0:
                        # slab-0 halves of all four m-tiles run first: a
                        # ~2.6us PE runway while slab 1 is still streaming
                        if m == 0:
                            # slab-0 DATA matmuls of all m-tiles first (only
                            # s0 needed), one-hot matmuls after (ylr lands
                            # while they run)
                            for m_ in range(MT):
                                c0_pts[m_] = psum.tile(
                                    [128, 1024], F32, name="pt", tag="pt"
                                )
                                emit_region_data(c0_pts[m_], 0, m_, 0)
                            for m_ in range(MT):
                                emit_region_oh(c0_pts[m_], 0, m_, 0)
                        pt = c0_pts[m]
                        emit_region(pt, 0, m, 1)
                    else:
                        pt = psum.tile([128, 1024], F32, name="pt", tag="pt")
                        for sj in range(2):
                            emit_region(pt, c, m, sj)
                    # hardest-negative reduction, split by chunk route
                    et = None
                    if (c, m) in ACT_MAX_CHUNKS:
                        # ACT: e = 2 - P/32 = dist^2 + 4S to fp16 SBUF;
                        # DVE folds it into the m-tile's TT-min chain at
                        # 2 elem/cycle, one 1x final reduce per m-tile
                        et = esbp.tile([128, 1024], F16, name="et", tag="et")
                        nc.scalar.activation(
                            et[:], pt[:], ACTF.Relu,
                            bias=bias_c["p2"][:], scale=-1.0 / 32.0,
                        )
                        if chain[m] is None:
                            chain[m] = et
                        else:
                            r = chnp.tile([128, 1024], F16, name="rc", tag="rc")
                            nc.vector.tensor_tensor(
                                r[:], chain[m][:], et[:], ALU.min
                            )
                            chain[m] = r
                        if c == last_act_c[m]:
                            nc.vector.tensor_reduce(
                                parts[:, 4 * m : 4 * m + 1], chain[m][:],
                                axis=AXX, op=ALU.min,
                            )
                    else:
                        # DVE direct: rowmax(P) from PSUM.  The last two
                        # chunks reduce in 512-col halves so the slab-6
                        # half overlaps the slab-7 matmuls and only one
                        # short reduce trails the final matmul.
                        mcol = 4 * m + c
                        if c == 3:
                            extra = nstat - 4 + m
                            nc.vector.tensor_reduce(
                                parts[:, mcol : mcol + 1], pt[:, 0:512],
                                axis=AXX, op=ALU.max,
                            )
                            nc.vector.tensor_reduce(
                                parts[:, extra : extra + 1], pt[:, 512:1024],
                                axis=AXX, op=ALU.max,
                            )
                        else:
                            nc.vector.tensor_reduce(
                                parts[:, mcol : mcol + 1], pt[:],
                                axis=AXX, op=ALU.max,
                            )
                    # positive pass: dist = sqrt(relu(-P/32 - 2)) over the
                    # diagonal window (= sqrt(relu(e - 4)) on the e-path);
                    # accum_out emits the row-sum for free
                    for (m_, c_, lo, hi) in segs:
                        if m_ != m or c_ != c:
                            continue
                        w = hi - lo
                        col = segcol[(m_, c_, lo, hi)]
                        if et is not None:
                            nc.scalar.activation(
                                wt[:, :w], et[:, lo:hi], ACTF.Relu,
                                bias=bias_c["m4"][:],
                            )
                        else:
                            nc.scalar.activation(
                                wt[:, :w], pt[:, lo:hi], ACTF.Relu,
                                bias=bias_c["m2"][:], scale=-1.0 / 32.0,
                            )
                        nc.scalar.activation(
                            dsc[:, :w], wt[:, :w], ACTF.Sqrt,
                            bias=bias_c["z"][:],
                            accum_out=parts[:, col : col + 1],
                        )

            nc.scalar.dma_start(stats_d.ap(), parts[:])

    nc.compile()
    return nc, segs, oh_slabs, nstat


_NC_CACHE: dict = {}


def _get_nc(maxc):
    if maxc not in _NC_CACHE:
        _NC_CACHE[maxc] = _build_nc(maxc)
    return _NC_CACHE[maxc]


def _prep_inputs(embeddings: np.ndarray, labels: np.ndarray):
    E = np.asarray(embeddings, dtype=np.float32)
    L = np.asarray(labels).astype(np.int64)
    assert E.shape == (B, D) and L.shape == (B,)

    order = np.argsort(L, kind="stable")
    Ls = L[order]
    nrm = np.maximum(np.linalg.norm(E, axis=1), 1e-12)
    N = (E / nrm[:, None]).astype(np.float32)[order]

    cnt = np.bincount(Ls, minlength=C)
    maxc = int(cnt.max())
    nc, segs, oh_slabs, nstat = _get_nc(maxc)

    X8 = np.ascontiguousarray((SCALE * N).T.astype(NP_FP8))       # [D, B]
    # S[g][p][c][x] = X8[128c + p, 512g + x]
    S = np.ascontiguousarray(
        X8.reshape(KC, 128, NJ, 512).transpose(2, 1, 0, 3)
    )                                                             # [g,p,c,x]
    Y = (Ls[None, :] == np.arange(C, dtype=np.int64)[:, None]).astype(np.float32)

    in_maps = []
    for r in range(NCORES):
        Sr = np.roll(S, -r, axis=0)                               # local j
        rows = slice(SHARD * r, SHARD * (r + 1))
        ylc = np.ascontiguousarray(
            (2 * SCALE * Y[:, rows]).reshape(2, 32, SHARD)
            .transpose(1, 0, 2).astype(NP_FP8)
        )
        yrr = np.stack(
            [
                (-SCALE * Y[:, 512 * ((r + s) % NJ) : 512 * ((r + s) % NJ) + 512])
                .reshape(2, 32, 512)
                for s in oh_slabs
            ]
        )                                                         # [t,h,p,x]
        yrr = np.ascontiguousarray(yrr.transpose(2, 0, 1, 3).astype(NP_FP8))
        in_maps.append(
            {
                "atp0": np.ascontiguousarray(Sr[0]),
                "atp1": np.ascontiguousarray(Sr[1]),
                "atp23": np.ascontiguousarray(Sr[2:4].transpose(1, 0, 2, 3)),
                "atp4567": np.ascontiguousarray(
                    Sr[4:8].reshape(2, 2, 128, KC, 512).transpose(2, 0, 1, 3, 4)
                ),
                "ylr": np.ascontiguousarray(
                    np.concatenate([ylc[:, None, :, :], yrr], axis=1)
                ),
            }
        )

    pos_cnt = cnt[Ls] - 1
    neg_cnt = B - cnt[Ls]
    invc = (1.0 / np.maximum(pos_cnt, 1)).astype(np.float32)
    valid = ((pos_cnt > 0) & (neg_cnt > 0)).astype(np.float32)
    return nc, segs, nstat, in_maps, (invc, valid)


def _finish(results, segs, nstat, aux):
    invc, valid = aux
    pos_sum = np.empty(B, dtype=np.float32)
    neg2 = np.empty(B, dtype=np.float32)
    for r in range(NCORES):
        st = np.asarray(results[r]["stats"])                      # [128, nstat]
        grid = st[:, :16].reshape(128, MT, 4)
        act_ms = {m for (_, m) in ACT_MAX_CHUNKS}
        n2 = np.full((128, MT), np.inf, dtype=np.float32)
        for m in range(MT):
            if m in act_ms:
                # the m-tile's TT-min chain result lands in col 4m+0
                n2[:, m] = np.minimum(n2[:, m], grid[:, m, 0])
            for c in range(4):
                if (c, m) in ACT_MAX_CHUNKS or (c == 0 and m in act_ms):
                    continue
                n2[:, m] = np.minimum(n2[:, m], 2.0 - grid[:, m, c] / 32.0)
        for m in range(MT):
            extra = nstat - 4 + m
            n2[:, m] = np.minimum(n2[:, m], 2.0 - st[:, extra] / 32.0)
        ps = np.zeros((128, MT), dtype=np.float32)
        for i, (m, c, lo, hi) in enumerate(segs):
            ps[:, m] += st[:, 16 + i]
        rows = slice(SHARD * r, SHARD * (r + 1))
        pos_sum[rows] = ps.T.reshape(SHARD)
        neg2[rows] = n2.T.reshape(SHARD)
    pos_stat = pos_sum * invc
    neg_stat = np.sqrt(np.maximum(neg2, 0.0), dtype=np.float32)
    per_row = np.maximum(pos_stat - neg_stat + MARGIN, 0.0) * valid
    n_valid = float(valid.sum())
    total = float(per_row.sum(dtype=np.float32))
    out = total / max(n_valid, 1.0) if n_valid > 0 else 0.0
    return np.array(out, dtype=np.float32)


def kernel(embeddings, labels, _run_kwargs=None):
    nc, segs, nstat, in_maps, aux = _prep_inputs(embeddings, labels)
    res = run_bass_kernel_spmd(
        nc, in_maps, core_ids=list(range(NCORES)), **(_run_kwargs or {})
    )
    out = _finish(res.results, segs, nstat, aux)
    if _run_kwargs:
        return out, res
    return out
